# revision 29
# baseline (speedup 1.0000x reference)
"""Trainium2 Bass kernel for nn_DSQGAttentionN (banded sparse attention).

Sharding: 8 cores = 2 batches x 4 head-groups (4 heads each), fp16
matmul pipeline with fp32 PSUM accumulation. ~130us/core (TimelineSim),
vs 257us baseline.

Design:
  - No mask identity-matmuls on PE: scores are raw q.k; exp(score) on
    ScalarE is multiplied by a precomputed exp(mask) tile on DVE
    (exp(s+m) == exp(s)*exp(m); masked entries have exp(m)==0).
  - Gate projection in fp8e4 with DoubleRow perf mode (4x fewer PE
    cycles); sigmoid errors are damped ~10x by sigmoid' at bias 2, so
    output error stays ~8e-3 vs the 2e-2 gate.
  - Gate sigmoid computed as 0.5*tanh(z/2)+0.5: Tanh shares the Exp
    activation table, eliminating 15 x 1283ns act-table swaps.
  - Software-pipelined emission: projection half-tiles for token chunk
    c+1 and epilogue tiles for chunk c-1 are interleaved between
    attention groups of chunk c; AV matmuls trail their scores by two
    groups so ScalarE exp / DVE mask-mult latency is hidden.
  - Softmax denominator: ones-column of V gives the sum in the AV psum
    tile; fast DVE reciprocal runs on a [65, 512] SBUF staging ring
    (custom DVE ops cannot read PSUM on hardware, and base partition
    must be 0/32/64 for the ones-matmul broadcast that follows).
  - Chunk-3 tail: the last head drains per 128-token block and per-tci
    epilogues are emitted as soon as their dependencies close.
  - Input DMAs ordered so the first projection waits on ~1MB, not 8MB;
    y DMAs merged to [128, 1024] to halve HWDGE holds.
Host: sums the 4 head-group partials per batch, adds bout.
"""

import numpy as np

import concourse.bass as bass
import concourse.mybir as mybir
import concourse.tile as tile
from concourse import bacc
from concourse.bass_utils import run_bass_kernel_spmd
from concourse.dve_ops import RECIP_APPROX_FAST_CONSTS, RECIPROCAL_APPROX_FAST

F32 = mybir.dt.float32
F16 = mybir.dt.float16
F8 = mybir.dt.float8e4

B, N, D, H = 2, 2048, 1024, 16
HD = D // H
HG = 4            # heads per core
NB = N // 128     # 16 query blocks
G = [0, 1, 2, 3, 4, 6, 8, 12]   # relative key chunks that contain taps
OFFSETS = sorted(set(range(0, 33)) | {48, 64, 96, 128, 192, 256, 384, 512, 768, 1024, 1536})
MASK_NEG = -30000.0
EXP_SHIFT = -3.0   # folded into exp(mask); keeps exp(score) small in fp16

ADD = mybir.AluOpType.add
MULT = mybir.AluOpType.mult
EXP = mybir.ActivationFunctionType.Exp
IDENT = mybir.ActivationFunctionType.Identity
SIGMOID = mybir.ActivationFunctionType.Sigmoid
TANH = mybir.ActivationFunctionType.Tanh


def build_nc():
    nc = bacc.Bacc("TRN2", target_bir_lowering=False, debug=False)

    xT = nc.dram_tensor("xT", [128, 8, N], F16, kind="ExternalInput")
    wqk = nc.dram_tensor("wqk", [128, 8, 512], F16, kind="ExternalInput")
    wv = nc.dram_tensor("wv", [128, 8, 256], F16, kind="ExternalInput")
    wg8 = nc.dram_tensor("wg8", [128, 8, 256], F8, kind="ExternalInput")
    xT8 = nc.dram_tensor("xT8", [128, 8, N], F8, kind="ExternalInput")
    wo = nc.dram_tensor("wo", [128, 2, D], F16, kind="ExternalInput")
    expm = nc.dram_tensor("expm", [128, HG, len(G), 128], F16, kind="ExternalInput")
    bias2 = nc.dram_tensor("bias2", [128, 8], F32, kind="ExternalInput")
    y = nc.dram_tensor("y", [N, D], F16, kind="ExternalOutput")

    with tile.TileContext(nc) as tc:
        with tc.tile_pool(name="persist", bufs=1) as persist:
            qkT = persist.tile([128, 4, N], F16)         # [part, (q01,q23,k01,k23), tok]
            vsb = persist.tile([128, NB, HG * 65], F16)  # V chunks; 65th col = ones
            gateT = persist.tile([128, 2, N], F16)
            wo_sb = persist.tile([128, 2, D], F16)
            expm_sb = persist.tile([128, HG, len(G), 128], F16)
            bias2_sb = persist.tile([128, 8], F32)
            onesb = persist.tile([128, 64], F16)
            avstage = persist.tile([65, HG, N], F16)     # rows 0-63 AV
            fgstage = persist.tile([128, 2, N], F16)
            # softmax denominator staging ring + reciprocal, held at
            # partition 64; rows 0-63 are memset filler so the DVE recip can
            # use the proven [0:65) partition range from SBUF F32.
            denbufs = [persist.tile([65, 512], F32, name=f"denbuf{i}")
                       for i in range(4)]
            reciprow = persist.tile([65, HG, N], F16)
            fgfinal = persist.tile([128, 2, N], F16)
            xT_sb = persist.tile([128, 8, N], F16)
            xT8_sb = persist.tile([128, 8, N], F8)
            wqk_sb = persist.tile([128, 8, 512], F16)
            wv_sb = persist.tile([128, 8, 256], F16)
            wg8_sb = persist.tile([128, 8, 256], F8)

            # input DMAs, ordered so chunk-0 projections can start early
            nc.sync.dma_start(out=xT_sb[:, :, 0:256], in_=xT.ap()[:, :, 0:256])
            nc.sync.dma_start(out=wqk_sb[:, :, 0:256], in_=wqk.ap()[:, :, 0:256])
            nc.sync.dma_start(out=xT_sb[:, :, 256:512], in_=xT.ap()[:, :, 256:512])
            nc.sync.dma_start(out=wqk_sb[:, :, 256:512], in_=wqk.ap()[:, :, 256:512])
            nc.sync.dma_start(out=bias2_sb, in_=bias2.ap())
            nc.sync.dma_start(out=wv_sb, in_=wv.ap())
            nc.sync.dma_start(out=wg8_sb, in_=wg8.ap())
            nc.sync.dma_start(out=xT8_sb[:, :, 0:512], in_=xT8.ap()[:, :, 0:512])
            nc.sync.dma_start(out=expm_sb, in_=expm.ap())
            nc.sync.dma_start(out=xT_sb[:, :, 512:1024], in_=xT.ap()[:, :, 512:1024])
            nc.sync.dma_start(out=xT8_sb[:, :, 512:1024], in_=xT8.ap()[:, :, 512:1024])
            nc.sync.dma_start(out=xT_sb[:, :, 1024:1536], in_=xT.ap()[:, :, 1024:1536])
            nc.sync.dma_start(out=xT8_sb[:, :, 1024:2048], in_=xT8.ap()[:, :, 1024:2048])
            nc.sync.dma_start(out=xT_sb[:, :, 1536:2048], in_=xT.ap()[:, :, 1536:2048])
            nc.sync.dma_start(out=wo_sb, in_=wo.ap())
            nc.vector.memset(onesb, 1.0)
            for db in denbufs:
                nc.gpsimd.memset(db, 1.0)
            for h in range(HG):
                nc.vector.memset(vsb[:, :, 65 * h + 64:65 * h + 65], 1.0)

            with (
                tc.tile_pool(name="psproj", bufs=2, space="PSUM") as psproj,
                tc.tile_pool(name="psst", bufs=2, space="PSUM") as psst,
                tc.tile_pool(name="psav", bufs=2, space="PSUM") as psav,
                tc.tile_pool(name="dpool", bufs=5) as dpool,
                tc.tile_pool(name="ypool", bufs=3) as ypool,
            ):
                # ---------- projection tile closures ----------
                # each tile is split into two emission parts so injected PE
                # work interleaves finely with attention matmuls
                def mk_projA(c, gi):
                    box = {}

                    def f1():
                        box["ps"] = psproj.tile([128, 512], F32, tag="proj",
                                                name="ps")
                        for kc in range(4):
                            nc.tensor.matmul(
                                box["ps"],
                                lhsT=wqk_sb[:, kc, gi * 128:(gi + 1) * 128],
                                rhs=xT_sb[:, kc, c * 512:(c + 1) * 512],
                                start=(kc == 0), stop=False,
                            )

                    def f2():
                        ps = box["ps"]
                        for kc in range(4, 8):
                            nc.tensor.matmul(
                                ps,
                                lhsT=wqk_sb[:, kc, gi * 128:(gi + 1) * 128],
                                rhs=xT_sb[:, kc, c * 512:(c + 1) * 512],
                                start=False, stop=(kc == 7),
                            )
                        nc.vector.tensor_scalar(
                            qkT[:, gi, c * 512:(c + 1) * 512], ps,
                            (HD ** -0.5) if gi < 2 else 1.0,
                            bias2_sb[:, gi:gi + 1],
                            op0=MULT, op1=ADD,
                        )
                    return [f1, f2]

                def mk_projB(c, half):
                    base = 4 * c + 2 * half
                    box = {}

                    def f1():
                        box["psv"] = psproj.tile([128, 512], F32, tag="proj",
                                                 name="psv")
                        for kc in range(8):
                            nc.tensor.matmul(
                                box["psv"][:, 0:256],
                                lhsT=xT_sb[:, kc, base * 128:(base + 1) * 128],
                                rhs=wv_sb[:, kc, :],
                                start=(kc == 0), stop=(kc == 7),
                                skip_group_check=True,
                            )

                    def f2():
                        psv = box["psv"]
                        for kc in range(8):
                            nc.tensor.matmul(
                                psv[:, 256:512],
                                lhsT=xT_sb[:, kc, (base + 1) * 128:(base + 2) * 128],
                                rhs=wv_sb[:, kc, :],
                                start=(kc == 0), stop=(kc == 7),
                                skip_group_check=True,
                            )
                        nc.vector.tensor_scalar(
                            vsb[:, base:base + 2, :].rearrange(
                                "p t (h u) -> p t h u", u=65)[:, :, :, 0:64],
                            psv.rearrange("p (t h u) -> p t h u", t=2, u=64),
                            0.0, None, op0=ADD,
                        )
                    return [f1, f2]

                def mk_projC(c, gi2):
                    def f():
                        psg = psproj.tile([128, 512], F32, tag="proj")
                        for kc2 in range(4):
                            nc.tensor.matmul(
                                psg,
                                lhsT=wg8_sb[:, 2 * kc2:2 * kc2 + 2,
                                            gi2 * 128:(gi2 + 1) * 128],
                                rhs=xT8_sb[:, 2 * kc2:2 * kc2 + 2,
                                           c * 512:(c + 1) * 512],
                                start=(kc2 == 0), stop=(kc2 == 3),
                                perf_mode=mybir.MatmulPerfMode.DoubleRow,
                            )
                        # sigmoid(z+bg) = 0.5*tanh((z+bg)/2) + 0.5; Tanh lives
                        # in the same act table as Exp (no table thrash)
                        gt = dpool.tile([128, 512], F16, tag="gt")
                        nc.scalar.activation(
                            gt, psg, TANH,
                            bias=bias2_sb[:, 4 + gi2:5 + gi2], scale=0.5,
                        )
                        nc.vector.tensor_scalar(
                            gateT[:, gi2, c * 512:(c + 1) * 512], gt,
                            0.5, 0.5, op0=MULT, op1=ADD,
                        )
                    return f

                def proj_tiles(c):
                    out = []
                    for mk in (mk_projA(c, 0), mk_projA(c, 1), mk_projB(c, 0),
                               mk_projA(c, 2), mk_projA(c, 3), mk_projB(c, 1)):
                        out.extend(mk)
                    out.append(mk_projC(c, 0))
                    out.append(mk_projC(c, 1))
                    return out

                # ---------- attention group closures ----------
                av_state = {}

                def mk_scores(h, qb, ref):
                    def f():
                        pq = 64 * (h % 2)
                        pg = h // 2
                        gs = [g for g in G if qb - g >= 0]
                        ngs = len(gs)
                        st = psst.tile([128, len(G) * 128], F32, tag="st")
                        for gi, g in enumerate(gs):
                            m = qb - g
                            nc.tensor.matmul(
                                st[:, gi * 128:(gi + 1) * 128],
                                lhsT=qkT[pq:pq + 64, 2 + pg, m * 128:(m + 1) * 128],
                                rhs=qkT[pq:pq + 64, pg, qb * 128:(qb + 1) * 128],
                                start=True, stop=True, skip_group_check=True,
                            )
                        expst = dpool.tile([128, len(G), 128], F16, tag="expst")
                        nc.scalar.activation(
                            expst[:, 0:ngs, :],
                            st[:, 0:ngs * 128].rearrange(
                                "p (a b) -> p a b", b=128),
                            EXP,
                        )
                        mst = dpool.tile([128, len(G), 128], F16, tag="mst")
                        nc.vector.tensor_mul(
                            mst[:, 0:ngs, :], expst[:, 0:ngs, :],
                            expm_sb[:, h, 0:ngs, :])
                        ref[0] = mst
                    return f

                def mk_av(h, qb, ref):
                    def f():
                        pq = 64 * (h % 2)
                        pg = h // 2
                        gs = [g for g in G if qb - g >= 0]
                        ngs = len(gs)
                        qs = qb % 4
                        qbg = qb // 4
                        if qs == 0:
                            av_state[h] = psav.tile([65, 512], F32, tag="av", name="av")
                        av = av_state[h]
                        mst = ref[0]
                        for gi, g in enumerate(gs):
                            m = qb - g
                            nc.tensor.matmul(
                                av[:, qs * 128:(qs + 1) * 128],
                                lhsT=vsb[:, m, 65 * h:65 * h + 65],
                                rhs=mst[:, gi, :],
                                start=(gi == 0), stop=(gi == ngs - 1),
                                skip_group_check=True,
                            )
                        if qbg == 3 and h == 3:
                            # final head: drain each 128-token block as soon as
                            # its AV accumulation closes
                            slq = slice(qbg * 512 + qs * 128,
                                        qbg * 512 + (qs + 1) * 128)
                            cq = slice(qs * 128, (qs + 1) * 128)
                            nc.scalar.copy(avstage[0:64, h, slq], av[0:64, cq])
                            nc.sync.dma_start(
                                out=fgstage[64:128, pg, slq],
                                in_=avstage[0:64, h, slq])
                            db = denbufs[h]
                            nc.vector.tensor_scalar(
                                db[64:65, cq], av[64:65, cq], 0.0, None, op0=ADD)
                            _c = RECIP_APPROX_FAST_CONSTS
                            nc.vector._custom_dve(
                                RECIPROCAL_APPROX_FAST,
                                out=reciprow[0:65, h, slq],
                                in0=db[0:65, cq],
                                s0=_c["s0"], s1=_c["s1"], imm2=_c["imm2"],
                            )
                        elif qs == 3:
                            sl = slice(qbg * 512, (qbg + 1) * 512)
                            if pq == 0:
                                nc.vector.tensor_scalar(
                                    fgstage[0:64, pg, sl], av[0:64, :],
                                    0.0, None, op0=ADD)
                            else:
                                nc.vector.tensor_scalar(
                                    avstage[0:64, h, sl], av[0:64, :],
                                    0.0, None, op0=ADD)
                                nc.sync.dma_start(
                                    out=fgstage[64:128, pg, sl],
                                    in_=avstage[0:64, h, sl])
                            db = denbufs[h]
                            nc.scalar.copy(db[64:65, :], av[64:65, :])
                            _c = RECIP_APPROX_FAST_CONSTS
                            nc.vector._custom_dve(
                                RECIPROCAL_APPROX_FAST,
                                out=reciprow[0:65, h, sl],
                                in0=db[0:65, :],
                                s0=_c["s0"], s1=_c["s1"], imm2=_c["imm2"],
                            )
                    return f

                # ---------- epilogue closures (per token chunk) ----------
                def mk_fgmul(c, pg):
                    def f():
                        sl = slice(c * 512, (c + 1) * 512)
                        rb = psproj.tile([128, 512], F32, tag="proj")
                        for half in range(2):
                            hh = 2 * pg + half
                            nc.tensor.matmul(
                                rb[64 * half:64 * half + 64, :],
                                lhsT=onesb[64:65, 0:64],
                                rhs=reciprow[64:65, hh, sl],
                                start=True, stop=True, skip_group_check=True,
                            )
                        tmp = ypool.tile([128, 512], F16, tag="tmp")
                        nc.vector.tensor_mul(tmp, fgstage[:, pg, sl], rb)
                        nc.vector.scalar_tensor_tensor(
                            out=fgfinal[:, pg, sl],
                            in0=tmp,
                            scalar=bias2_sb[:, 6 + pg:7 + pg],
                            in1=gateT[:, pg, sl],
                            op0=ADD, op1=MULT,
                        )
                    return f

                def mk_outproj(c, t2):
                    tci = 4 * c + t2

                    def f():
                        ysb = ypool.tile([128, 1024], F16, tag="y")
                        for nt2 in range(2):
                            psy = psproj.tile([128, 512], F32, tag="proj")
                            for kc2 in range(2):
                                nc.tensor.matmul(
                                    psy,
                                    lhsT=fgfinal[:, kc2, tci * 128:(tci + 1) * 128],
                                    rhs=wo_sb[:, kc2, nt2 * 512:(nt2 + 1) * 512],
                                    start=(kc2 == 0), stop=(kc2 == 1),
                                )
                            if (t2 + nt2) % 2 == 0:
                                nc.scalar.copy(
                                    ysb[:, nt2 * 512:(nt2 + 1) * 512], psy)
                            else:
                                nc.vector.tensor_scalar(
                                    ysb[:, nt2 * 512:(nt2 + 1) * 512], psy,
                                    0.0, None, op0=ADD)
                        nc.sync.dma_start(
                            out=y.ap()[tci * 128:(tci + 1) * 128, :], in_=ysb)
                    return f

                def epi_tiles(c):
                    out = [mk_fgmul(c, 0), mk_fgmul(c, 1)]
                    for t2 in range(4):
                        out.append(mk_outproj(c, t2))
                    return out

                def mk_epi3(t2):
                    # chunk-3 tail: per-128-token fgmul (pg=1) + out-proj,
                    # emitted as soon as head 3 drains that block
                    tci = 12 + t2

                    def f():
                        slq = slice(tci * 128, (tci + 1) * 128)
                        pg = 1
                        rb = psproj.tile([128, 512], F32, tag="proj")
                        for half in range(2):
                            hh = 2 * pg + half
                            nc.tensor.matmul(
                                rb[64 * half:64 * half + 64, 0:128],
                                lhsT=onesb[64:65, 0:64],
                                rhs=reciprow[64:65, hh, slq],
                                start=True, stop=True, skip_group_check=True,
                            )
                        tmp = ypool.tile([128, 512], F16, tag="tmp")
                        nc.vector.tensor_mul(
                            tmp[:, 0:128], fgstage[:, pg, slq], rb[:, 0:128])
                        nc.vector.scalar_tensor_tensor(
                            out=fgfinal[:, pg, slq],
                            in0=tmp[:, 0:128],
                            scalar=bias2_sb[:, 6 + pg:7 + pg],
                            in1=gateT[:, pg, slq],
                            op0=ADD, op1=MULT,
                        )
                        mk_outproj(3, t2)()
                    return f

                # ---------- emission: interleaved schedule ----------
                # chunk 0 qk-projection at 256-token granularity so the first
                # matmul only waits on a quarter of the startup DMA bytes
                for gi in range(4):
                    for half in range(2):
                        ps = psproj.tile([128, 512], F32, tag="proj",
                                         name="ps0")
                        t0 = half * 256
                        for kc in range(8):
                            nc.tensor.matmul(
                                ps[:, 0:256],
                                lhsT=wqk_sb[:, kc, gi * 128:(gi + 1) * 128],
                                rhs=xT_sb[:, kc, t0:t0 + 256],
                                start=(kc == 0), stop=(kc == 7),
                                skip_group_check=True,
                            )
                        nc.scalar.activation(
                            qkT[:, gi, t0:t0 + 256], ps[:, 0:256], IDENT,
                            bias=bias2_sb[:, gi:gi + 1],
                            scale=(HD ** -0.5) if gi < 2 else 1.0,
                        )
                for f in (*mk_projB(0, 0), *mk_projB(0, 1),
                          mk_projC(0, 0), mk_projC(0, 1)):
                    f()

                for c in range(4):
                    injects = []
                    if c + 1 < 4:
                        injects.extend(proj_tiles(c + 1))
                    if c >= 1:
                        injects.extend(epi_tiles(c - 1))
                    units = []
                    for h in range(HG):
                        for qs in range(4):
                            ref = [None]
                            units.append((mk_scores(h, 4 * c + qs, ref),
                                          mk_av(h, 4 * c + qs, ref)))
                    last = (c == 3)
                    pending = []
                    ninj = len(injects)
                    taken = 0
                    for i, (sc, avf) in enumerate(units):
                        sc()
                        # front-load injects on the last chunk to free the tail
                        rate = 2 if last else 1
                        want = min(ninj, (i + 1) * ninj * rate // len(units))
                        while taken < want:
                            injects[taken]()
                            taken += 1
                        pending.append(avf)
                        if len(pending) > 2:
                            pending.pop(0)()
                        if last and i == 9:
                            # heads 0,1 of chunk 3 drained -> first half of
                            # the final epilogue can start
                            mk_fgmul(3, 0)()
                        if last and i >= 14:
                            pending.pop(0)()
                            mk_epi3(i - 14)()
                    for avf in pending:
                        avf()
                    while taken < ninj:
                        injects[taken]()
                        taken += 1

                mk_epi3(2)()
                mk_epi3(3)()

    nc.compile()
    return nc


def make_core_inputs(inputs, b, hg):
    x = np.asarray(inputs["x"], np.float32)
    Wqkv = np.asarray(inputs["Wqkv"], np.float32)
    bqkv = np.asarray(inputs["bqkv"], np.float32)
    Wgate = np.asarray(inputs["Wgate"], np.float32)
    bgate = np.asarray(inputs["bgate"], np.float32)
    Wout = np.asarray(inputs["Wout"], np.float32)
    pos_bias = np.asarray(inputs["pos_bias"], np.float32)

    H0 = HG * hg
    xT = np.ascontiguousarray(x[b].T).reshape(8, 128, N).transpose(1, 0, 2)

    cols = []
    for base in (0, D):   # q then k
        for hp in range(2):
            for hh in range(2):
                hglob = H0 + 2 * hp + hh
                cols.append(np.arange(base + 64 * hglob, base + 64 * hglob + 64))
    cols = np.concatenate(cols)
    wqk = Wqkv[:, cols].reshape(8, 128, 512).transpose(1, 0, 2)
    bqk2 = np.ascontiguousarray(bqkv[cols].reshape(4, 128).T)

    vcols = np.arange(2 * D + 64 * H0, 2 * D + 64 * H0 + 256)
    wv = Wqkv[:, vcols].reshape(8, 128, 256).transpose(1, 0, 2)
    bv2 = np.ascontiguousarray(bqkv[vcols].reshape(2, 128).T)

    gcols = np.arange(256 * hg, 256 * hg + 256)
    wg = Wgate[:, gcols].reshape(8, 128, 256).transpose(1, 0, 2)
    bg2 = np.ascontiguousarray(bgate[gcols].reshape(2, 128).T) * 0.5

    wo = Wout[256 * hg:256 * hg + 256, :].reshape(2, 128, D).transpose(1, 0, 2)

    off_idx = {d: i for i, d in enumerate(OFFSETS)}
    jj = np.arange(128)[:, None]
    ii = np.arange(128)[None, :]
    maskt = np.full((128, HG, len(G), 128), MASK_NEG, np.float32)
    for gi, g in enumerate(G):
        delta = 128 * g + ii - jj
        base_m = np.full((128, 128), MASK_NEG, np.float32)
        sels = [(delta == dlt, oi) for dlt, oi in off_idx.items() if
                -127 <= dlt - 128 * g <= 127]
        for hl in range(HG):
            m = base_m.copy()
            for sel, oi in sels:
                m[sel] = pos_bias[oi, H0 + hl] + EXP_SHIFT
            maskt[:, hl, gi, :] = m
    expm = np.exp(maskt)  # masked entries -> exactly 0

    import ml_dtypes
    f16c = lambda a: np.ascontiguousarray(a, np.float16)
    f8c = lambda a: np.ascontiguousarray(
        np.asarray(a, np.float32), ml_dtypes.float8_e4m3)
    bias2 = np.concatenate([bqk2, bg2, bv2], axis=1).astype(np.float32)
    return dict(
        xT=f16c(xT), xT8=f8c(xT), wqk=f16c(wqk), wv=f16c(wv), wg8=f8c(wg),
        wo=f16c(wo), expm=f16c(expm), bias2=np.ascontiguousarray(bias2),
    )


_CACHE = {}


def _get_nc():
    if "nc" not in _CACHE:
        _CACHE["nc"] = build_nc()
    return _CACHE["nc"]


def kernel(**inputs):
    nc = _get_nc()
    in_maps = [make_core_inputs(inputs, c // 4, c % 4) for c in range(8)]
    res = run_bass_kernel_spmd(nc, in_maps, core_ids=list(range(8)))
    bout = np.asarray(inputs["bout"], np.float32)
    out = np.zeros((B, N, D), np.float32)
    for c in range(8):
        out[c // 4] += res.results[c]["y"].astype(np.float32)
    out += bout
    return out
